# revision 9
# baseline (speedup 1.0000x reference)
"""ContinualCLora forward on 8 TRN2 NeuronCores — single fused launch.

out = input @ W.T + bmask * sum_k gate_k * (input @ down[I_k] @ up[I_k])

Strategy (data-parallel on tokens, hint-compliant):
  - Each core gets 2048 tokens: 1024 from batches {0,1} (no delta) and 1024
    from batches {2,3} (delta applied), so every core runs the identical
    program and the batch mask is free.
  - The host hands x over TRANSPOSED ([DIN, tokens], still f32) so the device
    needs no DMA transposes at all: contraction dim lands on partitions
    directly off the cast-DMA (f32 -> bf16).
  - One launch does everything:
      * stream x in 8 chunk x 4 quarter pieces [128, 512], cast to bf16,
        kept resident in SBUF (4 MiB);
      * DVE partial-reduces each piece to build the per-core token sum
        s[128, 8] while loading;
      * 4 KiB DRAM AllReduce combines s across the 8 cores; omega = route.T @
        (s_tot/BS) on PE; the reference's top-k(+softmax, direct pool
        indexing) gating is replicated on-device with a min-mask trick
        (top-3-of-4 == drop the min), producing a [40,1] per-partition gate
        that scales the concatenated lora_up;
      * per 128-token tile: y = x @ W.T accumulated over 8 chunks in PSUM,
        held tiles additionally accumulate the rank-40 gated delta into the
        same PSUM banks before the store. Free tiles run first so the PE
        never waits on the collective.
"""

import json as _json

import ml_dtypes
import numpy as np

import concourse.bass as bass
import concourse.mybir as mybir
from concourse.bass import ts
from concourse.bass_utils import run_bass_kernel_spmd
from concourse.tile import TileContext
from concourse.vector_clock import ScopedClock

N_CORES = 8
B, S, DIN, DOUT = 4, 4096, 1024, 1024
POOL, R, TOPK, NUM_TASKS = 5, 8, 3, 5
T_CORE = (B * S) // N_CORES          # 2048 tokens per core
NT = T_CORE // 128                   # 16 tiles of 128 tokens
KC = DIN // 128                      # 8 contraction chunks
NQ = 2                               # load halves (1024 tokens each, 2KB lines)
QT = T_CORE // NQ
R5 = POOL * R                        # 40 concatenated lora rows
BF16 = ml_dtypes.bfloat16
AF = mybir.ActivationFunctionType
ALU = mybir.AluOpType
BIG = 245760.0                       # 30 * 8192: exp((om - BIG)/8192) ~ 0

# ---------------------------------------------------------------------------
# Workarounds for this walrus build: at most ONE sync wait per instruction
# (zero on DmaTransposeAnt).  Excess waits are hoisted onto standalone
# EventSemaphore instructions; the Tile exit drain gets its waits emitted as
# separate wait_ge ops.
# ---------------------------------------------------------------------------

_ZERO_WAIT_OPS = {"DmaTransposeAnt"}


def _fixup_bir(bir_bytes):
    bir = _json.loads(bir_bytes)
    n = 0
    for f in bir["functions"]:
        for blk in f["blocks"]:
            out = []
            for inst in blk["instructions"]:
                si = inst.get("sync_info")
                waits = (si or {}).get("on_wait") or []
                cap = 0 if inst.get("opcode") in _ZERO_WAIT_OPS else 1
                if len(waits) > cap:
                    for w in waits[cap:]:
                        n += 1
                        out.append({
                            "debug": inst.get("debug", 0),
                            "engine": inst["engine"],
                            "ins": [], "outs": [],
                            "name": f"{inst['name']}-xw{n}",
                            "opcode": "EventSemaphore",
                            "sync_info": {"on_update": [], "on_wait": [w]},
                        })
                    si["on_wait"] = waits[:cap]
                out.append(inst)
            blk["instructions"] = out
    return _json.dumps(bir).encode()


def _install_fixup(nc):
    orig = nc.to_json_bytes
    nc.to_json_bytes = lambda: _fixup_bir(orig())
    return nc


class _TC(TileContext):
    def _drain_and_barrier(self, tick_clock, wait_clock):
        probe = self.nc.sync.drain()
        wait_clock.add_sem_waits(probe.ins, ScopedClock({None: tick_clock.global_clock}))
        waits = [(w.ant_name, w.wait_value) for w in probe.ins.sync_info.on_wait]
        probe.ins.sync_info.on_wait = []
        name2sem = {v.name: v for v in self.sems.allocated().values()}
        for nm, val in waits:
            self.nc.sync.wait_ge(name2sem[nm], val)
        self.nc.sync.drain()
        self.nc.all_engine_barrier()
        popped = self.nc._tile_sem_poison_stack.pop()
        assert popped is self._sem_poison
        self.nc.clear_and_free_semaphores(list(self.sems.allocated().values()))
        self.nc.all_engine_barrier()


# ---------------------------------------------------------------------------
# Fused kernel: y = x @ W.T (+ gated low-rank delta on the second-half tiles)
# ---------------------------------------------------------------------------

def _build_kernel(n_sliced, k):
    """n_sliced = len(omega[1:tid+1]) candidates, k = experts kept."""
    rounds = n_sliced - k                 # how many smallest candidates to drop
    nc = bass.Bass(num_devices=N_CORES)
    x_d = nc.dram_tensor("x", [DIN, T_CORE], mybir.dt.bfloat16, kind="ExternalInput")
    wt_d = nc.dram_tensor("wt", [128, KC, DOUT], mybir.dt.bfloat16, kind="ExternalInput")
    dn_d = nc.dram_tensor("dn", [128, KC, R5], mybir.dt.bfloat16, kind="ExternalInput")
    up_d = nc.dram_tensor("up", [R5, DOUT], mybir.dt.bfloat16, kind="ExternalInput")
    rt_d = nc.dram_tensor("rt", [128, KC, POOL], mybir.dt.float32, kind="ExternalInput")
    km_d = nc.dram_tensor("km", [n_sliced, R5], mybir.dt.bfloat16, kind="ExternalInput")
    y_d = nc.dram_tensor("y", [T_CORE, DOUT], mybir.dt.bfloat16, kind="ExternalOutput")

    with _TC(nc) as tc:
        with (tc.tile_pool(name="cst", bufs=1) as cpool,
              tc.tile_pool(name="io", bufs=2) as io,
              tc.tile_pool(name="ys", bufs=3) as yo,
              tc.tile_pool(name="dram", bufs=1, space="DRAM") as dram,
              tc.tile_pool(name="gp", bufs=1, space="PSUM") as gps,
              tc.tile_pool(name="ps", bufs=2, space="PSUM") as ps):
            # ---- constants ----
            wt = cpool.tile([128, KC, DOUT], mybir.dt.bfloat16)
            for j in range(KC):
                nc.sync.dma_start(out=wt[:, j, :], in_=wt_d[:, j, :])
            dn = cpool.tile([128, KC, R5], mybir.dt.bfloat16)
            nc.sync.dma_start(out=dn[:], in_=dn_d[:])
            up = cpool.tile([R5, DOUT], mybir.dt.bfloat16)
            nc.sync.dma_start(out=up[:], in_=up_d[:])
            rt = cpool.tile([128, KC, POOL], mybir.dt.float32)
            nc.sync.dma_start(out=rt[:], in_=rt_d[:])
            km = cpool.tile([n_sliced, R5], mybir.dt.bfloat16)
            nc.sync.dma_start(out=km[:], in_=km_d[:])
            one1 = cpool.tile([1, 1], mybir.dt.bfloat16)
            nc.vector.memset(one1[:], 1.0)

            # ---- stream x in, cast to bf16, partial token sums on DVE ----
            xts = cpool.tile([128, KC, T_CORE], mybir.dt.bfloat16)
            sq = cpool.tile([128, KC, NQ], mybir.dt.float32)
            for q in range(NQ):
                for j in range(KC):
                    nc.gpsimd.dma_start(out=xts[:, j, ts(q, QT)],
                                        in_=x_d[ts(j, 128), ts(q, QT)])
                    nc.vector.tensor_reduce(out=sq[:, j, q:q + 1],
                                            in_=xts[:, j, ts(q, QT)],
                                            axis=mybir.AxisListType.X, op=ALU.add)
            s_sb = cpool.tile([128, KC], mybir.dt.float32)
            for j in range(KC):
                nc.vector.tensor_reduce(out=s_sb[:, j:j + 1], in_=sq[:, j, :],
                                        axis=mybir.AxisListType.X, op=ALU.add)

            # ---- 4 KiB AllReduce of the token sums (gpsimd is idle by now) ----
            cc_in = dram.tile([128, KC], mybir.dt.float32)
            cc_out = dram.tile([128, KC], mybir.dt.float32)
            nc.gpsimd.dma_start(out=cc_in[:], in_=s_sb[:])
            nc.gpsimd.collective_compute(
                "AllReduce", ALU.add,
                replica_groups=[list(range(N_CORES))],
                ins=[cc_in.opt()], outs=[cc_out.opt()],
            )
            s_tot = cpool.tile([128, KC], mybir.dt.float32)
            nc.gpsimd.dma_start(out=s_tot[:], in_=cc_out[:])

            # ---- main loop; gating emitted between free and held tiles ----
            g = gps.tile([R5, 16], mybir.dt.float32)     # om | wT | wrep slices
            up_s = cpool.tile([R5, DOUT], mybir.dt.bfloat16)

            for i in range(NT):
                held = i >= NT // 2  # tokens from batches {2,3}: apply delta

                if i == NT // 2:
                    # omega = (s_tot/BS) @ route  ->  g[0, :POOL] (PE)
                    for j in range(KC):
                        nc.tensor.matmul(g[0:1, 0:POOL], s_tot[:, j:j + 1],
                                         rt[:, j, :], start=(j == 0), stop=(j == KC - 1))
                    # replicate top-k-on-sliced gating: drop `rounds` smallest
                    oms = g[0:1, 1:1 + n_sliced]
                    excl = cpool.tile([1, n_sliced], mybir.dt.float32)
                    act = cpool.tile([1, n_sliced], mybir.dt.float32)
                    if rounds:
                        nc.vector.memset(excl[:], 0.0)
                        v = cpool.tile([1, n_sliced], mybir.dt.float32)
                        nc.vector.tensor_copy(v[:], oms)
                        for _ in range(rounds):
                            mn = cpool.tile([1, 1], mybir.dt.float32, tag="mn")
                            nc.vector.tensor_reduce(out=mn[:], in_=v[:],
                                                    axis=mybir.AxisListType.X, op=ALU.min)
                            mk = cpool.tile([1, n_sliced], mybir.dt.float32, tag="mk")
                            nc.vector.tensor_scalar(out=mk[:], in0=v[:], scalar1=mn[:],
                                                    scalar2=None, op0=ALU.is_equal)
                            nc.vector.tensor_tensor(out=excl[:], in0=excl[:], in1=mk[:],
                                                    op=ALU.add)
                            mkb = cpool.tile([1, n_sliced], mybir.dt.float32, tag="mkb")
                            nc.vector.tensor_scalar(out=mkb[:], in0=mk[:], scalar1=BIG,
                                                    scalar2=None, op0=ALU.mult)
                            nc.vector.tensor_tensor(out=v[:], in0=v[:], in1=mkb[:],
                                                    op=ALU.add)
                        exb = cpool.tile([1, n_sliced], mybir.dt.float32)
                        nc.vector.tensor_scalar(out=exb[:], in0=excl[:], scalar1=BIG,
                                                scalar2=None, op0=ALU.mult)
                        nc.vector.tensor_tensor(out=act[:], in0=oms, in1=exb[:],
                                                op=ALU.subtract)
                    else:
                        nc.vector.tensor_copy(act[:], oms)
                    # softmax over the survivors (dropped ones -> exp(-30) ~ 0)
                    e = cpool.tile([1, n_sliced], mybir.dt.float32)
                    nc.scalar.activation(e[:], act[:], AF.Exp, scale=1.0 / float(B * S))
                    ssum = cpool.tile([1, 1], mybir.dt.float32)
                    nc.vector.tensor_reduce(out=ssum[:], in_=e[:],
                                            axis=mybir.AxisListType.X, op=ALU.add)
                    rs = cpool.tile([1, 1], mybir.dt.float32)
                    nc.vector.reciprocal(rs[:], ssum[:])
                    w = cpool.tile([1, n_sliced], mybir.dt.bfloat16)
                    nc.vector.tensor_scalar(out=w[:], in0=e[:], scalar1=rs[:],
                                            scalar2=None, op0=ALU.mult)
                    # wrep[40,1] = Kmat.T @ w.T ; fold into lora_up rows
                    nc.tensor.matmul(g[0:n_sliced, 8:9], w[:], one1[:],
                                     start=True, stop=True)
                    wt4 = cpool.tile([n_sliced, 1], mybir.dt.bfloat16)
                    nc.vector.tensor_copy(wt4[:], g[0:n_sliced, 8:9])
                    nc.tensor.matmul(g[0:R5, 9:10], km[:], wt4[:],
                                     start=True, stop=True)
                    wrep = cpool.tile([R5, 1], mybir.dt.float32)
                    nc.vector.tensor_copy(wrep[:], g[0:R5, 9:10])
                    nc.scalar.activation(up_s[:], up[:], AF.Copy, scale=wrep[:])

                y = ps.tile([128, DOUT], mybir.dt.float32, tag="y")
                if held and i % 2 == 0:
                    # down-projection for the tile PAIR, ahead of the y matmuls
                    # so the DVE copy hides under them
                    pt = ps.tile([R5, 256], mybir.dt.float32, tag="pt")
                    for j in range(KC):
                        nc.tensor.matmul(pt[:], dn[:, j, :], xts[:, j, ts(i // 2, 256)],
                                         start=(j == 0), stop=(j == KC - 1))
                    pts = io.tile([R5, 256], mybir.dt.bfloat16, tag="pts")
                    nc.vector.tensor_copy(pts[:], pt[:])
                for j in range(KC):
                    last = (j == KC - 1) and not held
                    nc.tensor.matmul(y[:, 0:512], xts[:, j, ts(i, 128)],
                                     wt[:, j, 0:512], start=(j == 0), stop=last)
                    nc.tensor.matmul(y[:, 512:1024], xts[:, j, ts(i, 128)],
                                     wt[:, j, 512:1024], start=(j == 0), stop=last)
                if held:
                    ph = pts[:, ts(i % 2, 128)]
                    nc.tensor.matmul(y[:, 0:512], ph, up_s[:, 0:512],
                                     start=False, stop=True)
                    nc.tensor.matmul(y[:, 512:1024], ph, up_s[:, 512:1024],
                                     start=False, stop=True)
                ysb = yo.tile([128, DOUT], mybir.dt.bfloat16, tag="ysb")
                if held:
                    # DVE is past the load reduces by now; ACT handled the
                    # free tiles so those copies never queue behind reduces
                    nc.vector.tensor_copy(ysb[:], y[:])
                else:
                    nc.scalar.activation(ysb[:, 0:512], y[:, 0:512], AF.Copy)
                    nc.scalar.activation(ysb[:, 512:1024], y[:, 512:1024], AF.Copy)
                nc.sync.dma_start(out=y_d[ts(i, 128), :], in_=ysb[:])
    return _install_fixup(nc)


_NC_CACHE = {}


def _get_nc(n_sliced, k):
    key = (n_sliced, k)
    if key not in _NC_CACHE:
        _NC_CACHE[key] = _build_kernel(n_sliced, k)
    return _NC_CACHE[key]


LAST_RESULTS = {}  # test-harness hook: BassKernelResults of the last call


def kernel(input, W, lora_down, lora_up, lora_route, task_id):
    x = np.ascontiguousarray(np.asarray(input, dtype=np.float32)).reshape(B * S, DIN)
    W = np.asarray(W, dtype=np.float32)
    lora_down = np.asarray(lora_down, dtype=np.float32)
    lora_up = np.asarray(lora_up, dtype=np.float32)
    lora_route = np.asarray(lora_route, dtype=np.float32)
    tid = min(int(task_id), NUM_TASKS)
    k = min(tid, TOPK)

    half = (B * S) // 2
    per = half // N_CORES  # 1024 tokens from each half per core
    xT = np.ascontiguousarray(x.T).astype(BF16)  # [DIN, B*S]; the bf16 cast
    # replaces the cast-DMA the device would otherwise do on the same values
    shards = [np.concatenate([xT[:, c * per:(c + 1) * per],
                              xT[:, half + c * per:half + (c + 1) * per]], axis=1)
              for c in range(N_CORES)]

    down_cat = lora_down.transpose(1, 0, 2).reshape(DIN, R5)
    wt_h = np.ascontiguousarray(W.T.reshape(KC, 128, DOUT).transpose(1, 0, 2)).astype(BF16)
    dn_h = np.ascontiguousarray(down_cat.reshape(KC, 128, R5).transpose(1, 0, 2)).astype(BF16)
    up_h = lora_up.reshape(R5, DOUT).astype(BF16)
    rt_h = np.ascontiguousarray(
        lora_route[1].reshape(KC, 128, POOL).transpose(1, 0, 2)).astype(np.float32)
    km_h = np.zeros((tid, R5), np.float32)
    for p in range(min(tid, POOL)):
        km_h[p, p * R:(p + 1) * R] = 1.0  # sliced position p -> expert p

    in_maps = [{"x": s, "wt": wt_h, "dn": dn_h, "up": up_h,
                "rt": rt_h, "km": km_h.astype(BF16)} for s in shards]
    res = run_bass_kernel_spmd(_get_nc(tid, k), in_maps, list(range(N_CORES)))
    LAST_RESULTS["b"] = res

    y = np.empty((B * S, DOUT), np.float32)
    for c in range(N_CORES):
        yc = res.results[c]["y"].astype(np.float32)
        y[c * per:(c + 1) * per] = yc[:per]
        y[half + c * per:half + (c + 1) * per] = yc[per:]
    return y.reshape(B, S, DOUT)


# revision 12
# speedup vs baseline: 1.1278x; 1.1278x over previous
"""ContinualCLora forward on 8 TRN2 NeuronCores — single fused launch.

out = input @ W.T + bmask * sum_k gate_k * (input @ down[I_k] @ up[I_k])

Strategy (data-parallel on tokens, hint-compliant):
  - Each core gets 2048 tokens: 1024 from batches {0,1} (no delta) and 1024
    from batches {2,3} (delta applied), so every core runs the identical
    program and the batch mask is free.
  - The host hands x over TRANSPOSED ([DIN, tokens], still f32) so the device
    needs no DMA transposes at all: contraction dim lands on partitions
    directly off the cast-DMA (f32 -> bf16).
  - One launch does everything:
      * stream x in 8 chunk x 4 quarter pieces [128, 512], cast to bf16,
        kept resident in SBUF (4 MiB);
      * DVE partial-reduces each piece to build the per-core token sum
        s[128, 8] while loading;
      * 4 KiB DRAM AllReduce combines s across the 8 cores; omega = route.T @
        (s_tot/BS) on PE; the reference's top-k(+softmax, direct pool
        indexing) gating is replicated on-device with a min-mask trick
        (top-3-of-4 == drop the min), producing a [40,1] per-partition gate
        that scales the concatenated lora_up;
      * per 128-token tile: y = x @ W.T accumulated over 8 chunks in PSUM,
        held tiles additionally accumulate the rank-40 gated delta into the
        same PSUM banks before the store. Free tiles run first so the PE
        never waits on the collective.
"""

import json as _json

import ml_dtypes
import numpy as np

import concourse.bass as bass
import concourse.mybir as mybir
from concourse.bass import ts
from concourse.bass_utils import run_bass_kernel_spmd
from concourse.tile import TileContext
from concourse.vector_clock import ScopedClock

N_CORES = 8
B, S, DIN, DOUT = 4, 4096, 1024, 1024
POOL, R, TOPK, NUM_TASKS = 5, 8, 3, 5
T_CORE = (B * S) // N_CORES          # 2048 tokens per core
NT = T_CORE // 128                   # 16 tiles of 128 tokens
KC = DIN // 128                      # 8 contraction chunks
NQ = 2                               # load halves (1024 tokens each, 2KB lines)
QT = T_CORE // NQ
R5 = POOL * R                        # 40 concatenated lora rows
BF16 = ml_dtypes.bfloat16
AF = mybir.ActivationFunctionType
ALU = mybir.AluOpType
BIG = 245760.0                       # 30 * 8192: exp((om - BIG)/8192) ~ 0

# ---------------------------------------------------------------------------
# Workarounds for this walrus build: at most ONE sync wait per instruction
# (zero on DmaTransposeAnt).  Excess waits are hoisted onto standalone
# EventSemaphore instructions; the Tile exit drain gets its waits emitted as
# separate wait_ge ops.
# ---------------------------------------------------------------------------

_ZERO_WAIT_OPS = {"DmaTransposeAnt"}


def _fixup_bir(bir_bytes):
    bir = _json.loads(bir_bytes)
    n = 0
    for f in bir["functions"]:
        for blk in f["blocks"]:
            out = []
            for inst in blk["instructions"]:
                si = inst.get("sync_info")
                waits = (si or {}).get("on_wait") or []
                cap = 0 if inst.get("opcode") in _ZERO_WAIT_OPS else 1
                if len(waits) > cap:
                    for w in waits[cap:]:
                        n += 1
                        out.append({
                            "debug": inst.get("debug", 0),
                            "engine": inst["engine"],
                            "ins": [], "outs": [],
                            "name": f"{inst['name']}-xw{n}",
                            "opcode": "EventSemaphore",
                            "sync_info": {"on_update": [], "on_wait": [w]},
                        })
                    si["on_wait"] = waits[:cap]
                out.append(inst)
            blk["instructions"] = out
    return _json.dumps(bir).encode()


def _install_fixup(nc):
    orig = nc.to_json_bytes
    nc.to_json_bytes = lambda: _fixup_bir(orig())
    return nc


class _TC(TileContext):
    def _drain_and_barrier(self, tick_clock, wait_clock):
        probe = self.nc.sync.drain()
        wait_clock.add_sem_waits(probe.ins, ScopedClock({None: tick_clock.global_clock}))
        waits = [(w.ant_name, w.wait_value) for w in probe.ins.sync_info.on_wait]
        probe.ins.sync_info.on_wait = []
        name2sem = {v.name: v for v in self.sems.allocated().values()}
        for nm, val in waits:
            self.nc.sync.wait_ge(name2sem[nm], val)
        self.nc.sync.drain()
        self.nc.all_engine_barrier()
        popped = self.nc._tile_sem_poison_stack.pop()
        assert popped is self._sem_poison
        self.nc.clear_and_free_semaphores(list(self.sems.allocated().values()))
        self.nc.all_engine_barrier()


# ---------------------------------------------------------------------------
# Fused kernel: y = x @ W.T (+ gated low-rank delta on the second-half tiles)
# ---------------------------------------------------------------------------

def _build_kernel(n_sliced, k):
    """n_sliced = len(omega[1:tid+1]) candidates, k = experts kept."""
    rounds = n_sliced - k                 # how many smallest candidates to drop
    nc = bass.Bass(num_devices=N_CORES)
    x_d = nc.dram_tensor("x", [DIN, T_CORE], mybir.dt.bfloat16, kind="ExternalInput")
    wt_d = nc.dram_tensor("wt", [128, KC, DOUT], mybir.dt.bfloat16, kind="ExternalInput")
    dn_d = nc.dram_tensor("dn", [128, KC, R5], mybir.dt.bfloat16, kind="ExternalInput")
    up_d = nc.dram_tensor("up", [R5, DOUT], mybir.dt.bfloat16, kind="ExternalInput")
    rt_d = nc.dram_tensor("rt", [128, KC, POOL], mybir.dt.float32, kind="ExternalInput")
    km_d = nc.dram_tensor("km", [n_sliced, R5], mybir.dt.bfloat16, kind="ExternalInput")
    y_d = nc.dram_tensor("y", [T_CORE, DOUT], mybir.dt.bfloat16, kind="ExternalOutput")

    with _TC(nc) as tc:
        with (tc.tile_pool(name="cst", bufs=1) as cpool,
              tc.tile_pool(name="io", bufs=2) as io,
              tc.tile_pool(name="ys", bufs=3) as yo,
              tc.tile_pool(name="dram", bufs=1, space="DRAM") as dram,
              tc.tile_pool(name="gp", bufs=1, space="PSUM") as gps,
              tc.tile_pool(name="ps", bufs=2, space="PSUM") as ps):
            # ---- x first half + W interleaved on sync (earliest queue) so
            #      the PE can start tile 0 as the pieces stream in ----
            wt = cpool.tile([128, KC, DOUT], mybir.dt.bfloat16)
            xts = cpool.tile([128, KC, T_CORE], mybir.dt.bfloat16)
            for j in range(KC):
                nc.sync.dma_start(out=xts[:, j, ts(0, QT)],
                                  in_=x_d[ts(j, 128), ts(0, QT)])
                nc.sync.dma_start(out=wt[:, j, :], in_=wt_d[:, j, :])
            # ---- second half + small constants on gpsimd ----
            for j in range(KC):
                nc.gpsimd.dma_start(out=xts[:, j, ts(1, QT)],
                                    in_=x_d[ts(j, 128), ts(1, QT)])
            dn = cpool.tile([128, KC, R5], mybir.dt.bfloat16)
            nc.gpsimd.dma_start(out=dn[:], in_=dn_d[:])
            up = cpool.tile([R5, DOUT], mybir.dt.bfloat16)
            nc.gpsimd.dma_start(out=up[:], in_=up_d[:])
            rt = cpool.tile([128, KC, POOL], mybir.dt.float32)
            nc.gpsimd.dma_start(out=rt[:], in_=rt_d[:])
            km = cpool.tile([n_sliced, R5], mybir.dt.bfloat16)
            nc.gpsimd.dma_start(out=km[:], in_=km_d[:])
            one1 = cpool.tile([1, 1], mybir.dt.bfloat16)
            nc.vector.memset(one1[:], 1.0)

            # ---- partial token sums on the (otherwise idle) scalar engine;
            #      DVE stays free for the PSUM->SBUF output copies ----
            sq = cpool.tile([128, KC, NQ], mybir.dt.float32)
            scr = cpool.tile([128, QT], mybir.dt.bfloat16)
            for q in range(NQ):
                for j in range(KC):
                    nc.scalar.activation(scr[:], xts[:, j, ts(q, QT)], AF.Copy,
                                         accum_out=sq[:, j, q:q + 1])

            # ---- 8 KiB AllReduce of the piece sums (gpsimd is idle by now;
            #      halves are summed implicitly by the omega matmuls) ----
            cc_in = dram.tile([128, KC, NQ], mybir.dt.float32)
            cc_out = dram.tile([128, KC, NQ], mybir.dt.float32)
            nc.gpsimd.dma_start(out=cc_in[:], in_=sq[:])
            nc.gpsimd.collective_compute(
                "AllReduce", ALU.add,
                replica_groups=[list(range(N_CORES))],
                ins=[cc_in.opt()], outs=[cc_out.opt()],
            )
            s_tot = cpool.tile([128, KC, NQ], mybir.dt.float32)
            nc.gpsimd.dma_start(out=s_tot[:], in_=cc_out[:])

            # ---- main loop; gating emitted between free and held tiles ----
            g = gps.tile([R5, 16], mybir.dt.float32)     # om | wT | wrep slices
            up_s = cpool.tile([R5, DOUT], mybir.dt.bfloat16)

            for i in range(NT):
                held = i >= NT // 2  # tokens from batches {2,3}: apply delta

                if i == NT // 2:
                    # omega = (s_tot/BS) @ route  ->  g[0, :POOL] (PE)
                    for j in range(KC):
                        for q in range(NQ):
                            nc.tensor.matmul(g[0:1, 0:POOL], s_tot[:, j, q:q + 1],
                                             rt[:, j, :], start=(j == 0 and q == 0),
                                             stop=(j == KC - 1 and q == NQ - 1))
                    # replicate top-k-on-sliced gating: drop `rounds` smallest
                    oms = g[0:1, 1:1 + n_sliced]
                    excl = cpool.tile([1, n_sliced], mybir.dt.float32)
                    act = cpool.tile([1, n_sliced], mybir.dt.float32)
                    if rounds:
                        nc.vector.memset(excl[:], 0.0)
                        v = cpool.tile([1, n_sliced], mybir.dt.float32)
                        nc.vector.tensor_copy(v[:], oms)
                        for _ in range(rounds):
                            mn = cpool.tile([1, 1], mybir.dt.float32, tag="mn")
                            nc.vector.tensor_reduce(out=mn[:], in_=v[:],
                                                    axis=mybir.AxisListType.X, op=ALU.min)
                            mk = cpool.tile([1, n_sliced], mybir.dt.float32, tag="mk")
                            nc.vector.tensor_scalar(out=mk[:], in0=v[:], scalar1=mn[:],
                                                    scalar2=None, op0=ALU.is_equal)
                            nc.vector.tensor_tensor(out=excl[:], in0=excl[:], in1=mk[:],
                                                    op=ALU.add)
                            mkb = cpool.tile([1, n_sliced], mybir.dt.float32, tag="mkb")
                            nc.vector.tensor_scalar(out=mkb[:], in0=mk[:], scalar1=BIG,
                                                    scalar2=None, op0=ALU.mult)
                            nc.vector.tensor_tensor(out=v[:], in0=v[:], in1=mkb[:],
                                                    op=ALU.add)
                        exb = cpool.tile([1, n_sliced], mybir.dt.float32)
                        nc.vector.tensor_scalar(out=exb[:], in0=excl[:], scalar1=BIG,
                                                scalar2=None, op0=ALU.mult)
                        nc.vector.tensor_tensor(out=act[:], in0=oms, in1=exb[:],
                                                op=ALU.subtract)
                    else:
                        nc.vector.tensor_copy(act[:], oms)
                    # softmax over the survivors (dropped ones -> exp(-30) ~ 0)
                    e = cpool.tile([1, n_sliced], mybir.dt.float32)
                    nc.scalar.activation(e[:], act[:], AF.Exp, scale=1.0 / float(B * S))
                    ssum = cpool.tile([1, 1], mybir.dt.float32)
                    nc.vector.tensor_reduce(out=ssum[:], in_=e[:],
                                            axis=mybir.AxisListType.X, op=ALU.add)
                    rs = cpool.tile([1, 1], mybir.dt.float32)
                    nc.vector.reciprocal(rs[:], ssum[:])
                    w = cpool.tile([1, n_sliced], mybir.dt.bfloat16)
                    nc.vector.tensor_scalar(out=w[:], in0=e[:], scalar1=rs[:],
                                            scalar2=None, op0=ALU.mult)
                    # wrep[40,1] = Kmat.T @ w.T ; fold into lora_up rows
                    nc.tensor.matmul(g[0:n_sliced, 8:9], w[:], one1[:],
                                     start=True, stop=True)
                    wt4 = cpool.tile([n_sliced, 1], mybir.dt.bfloat16)
                    nc.vector.tensor_copy(wt4[:], g[0:n_sliced, 8:9])
                    nc.tensor.matmul(g[0:R5, 9:10], km[:], wt4[:],
                                     start=True, stop=True)
                    wrep = cpool.tile([R5, 1], mybir.dt.float32)
                    nc.vector.tensor_copy(wrep[:], g[0:R5, 9:10])
                    nc.scalar.activation(up_s[:], up[:], AF.Copy, scale=wrep[:])

                y = ps.tile([128, DOUT], mybir.dt.float32, tag="y")
                if held and i % 2 == 0:
                    # down-projection for the tile PAIR, ahead of the y matmuls
                    # so the DVE copy hides under them
                    pt = ps.tile([R5, 256], mybir.dt.float32, tag="pt")
                    for j in range(KC):
                        nc.tensor.matmul(pt[:], dn[:, j, :], xts[:, j, ts(i // 2, 256)],
                                         start=(j == 0), stop=(j == KC - 1))
                    pts = io.tile([R5, 256], mybir.dt.bfloat16, tag="pts")
                    nc.vector.tensor_copy(pts[:], pt[:])
                for j in range(KC):
                    last = (j == KC - 1) and not held
                    nc.tensor.matmul(y[:, 0:512], xts[:, j, ts(i, 128)],
                                     wt[:, j, 0:512], start=(j == 0), stop=last)
                    nc.tensor.matmul(y[:, 512:1024], xts[:, j, ts(i, 128)],
                                     wt[:, j, 512:1024], start=(j == 0), stop=last)
                if held:
                    ph = pts[:, ts(i % 2, 128)]
                    nc.tensor.matmul(y[:, 0:512], ph, up_s[:, 0:512],
                                     start=False, stop=True)
                    nc.tensor.matmul(y[:, 512:1024], ph, up_s[:, 512:1024],
                                     start=False, stop=True)
                ysb = yo.tile([128, DOUT], mybir.dt.bfloat16, tag="ysb")
                nc.vector.tensor_copy(ysb[:], y[:])
                nc.sync.dma_start(out=y_d[ts(i, 128), :], in_=ysb[:])
    return _install_fixup(nc)


_NC_CACHE = {}


def _get_nc(n_sliced, k):
    key = (n_sliced, k)
    if key not in _NC_CACHE:
        _NC_CACHE[key] = _build_kernel(n_sliced, k)
    return _NC_CACHE[key]


LAST_RESULTS = {}  # test-harness hook: BassKernelResults of the last call


def kernel(input, W, lora_down, lora_up, lora_route, task_id):
    x = np.ascontiguousarray(np.asarray(input, dtype=np.float32)).reshape(B * S, DIN)
    W = np.asarray(W, dtype=np.float32)
    lora_down = np.asarray(lora_down, dtype=np.float32)
    lora_up = np.asarray(lora_up, dtype=np.float32)
    lora_route = np.asarray(lora_route, dtype=np.float32)
    tid = min(int(task_id), NUM_TASKS)
    k = min(tid, TOPK)

    half = (B * S) // 2
    per = half // N_CORES  # 1024 tokens from each half per core
    xT = np.ascontiguousarray(x.T).astype(BF16)  # [DIN, B*S]; the bf16 cast
    # replaces the cast-DMA the device would otherwise do on the same values
    shards = [np.concatenate([xT[:, c * per:(c + 1) * per],
                              xT[:, half + c * per:half + (c + 1) * per]], axis=1)
              for c in range(N_CORES)]

    down_cat = lora_down.transpose(1, 0, 2).reshape(DIN, R5)
    wt_h = np.ascontiguousarray(W.T.reshape(KC, 128, DOUT).transpose(1, 0, 2)).astype(BF16)
    dn_h = np.ascontiguousarray(down_cat.reshape(KC, 128, R5).transpose(1, 0, 2)).astype(BF16)
    up_h = lora_up.reshape(R5, DOUT).astype(BF16)
    rt_h = np.ascontiguousarray(
        lora_route[1].reshape(KC, 128, POOL).transpose(1, 0, 2)).astype(np.float32)
    km_h = np.zeros((tid, R5), np.float32)
    for p in range(min(tid, POOL)):
        km_h[p, p * R:(p + 1) * R] = 1.0  # sliced position p -> expert p

    in_maps = [{"x": s, "wt": wt_h, "dn": dn_h, "up": up_h,
                "rt": rt_h, "km": km_h.astype(BF16)} for s in shards]
    res = run_bass_kernel_spmd(_get_nc(tid, k), in_maps, list(range(N_CORES)))
    LAST_RESULTS["b"] = res

    y = np.empty((B * S, DOUT), np.float32)
    for c in range(N_CORES):
        yc = res.results[c]["y"].astype(np.float32)
        y[c * per:(c + 1) * per] = yc[:per]
        y[half + c * per:half + (c + 1) * per] = yc[per:]
    return y.reshape(B, S, DOUT)


# revision 18
# speedup vs baseline: 1.1579x; 1.0266x over previous
"""ContinualCLora forward on 8 TRN2 NeuronCores — single fused launch.

out = input @ W.T + bmask * sum_k gate_k * (input @ down[I_k] @ up[I_k])

Strategy (data-parallel on tokens, hint-compliant):
  - Each core gets 2048 tokens: 1024 from batches {0,1} (no delta) and 1024
    from batches {2,3} (delta applied), so every core runs the identical
    program and the batch mask is free.
  - The host hands x over TRANSPOSED ([DIN, tokens], still f32) so the device
    needs no DMA transposes at all: contraction dim lands on partitions
    directly off the cast-DMA (f32 -> bf16).
  - One launch does everything:
      * stream x in 8 chunk x 4 quarter pieces [128, 512], cast to bf16,
        kept resident in SBUF (4 MiB);
      * DVE partial-reduces each piece to build the per-core token sum
        s[128, 8] while loading;
      * 4 KiB DRAM AllReduce combines s across the 8 cores; omega = route.T @
        (s_tot/BS) on PE; the reference's top-k(+softmax, direct pool
        indexing) gating is replicated on-device with a min-mask trick
        (top-3-of-4 == drop the min), producing a [40,1] per-partition gate
        that scales the concatenated lora_up;
      * per 128-token tile: y = x @ W.T accumulated over 8 chunks in PSUM,
        held tiles additionally accumulate the rank-40 gated delta into the
        same PSUM banks before the store. Free tiles run first so the PE
        never waits on the collective.
"""

import json as _json

import ml_dtypes
import numpy as np

import concourse.bass as bass
import concourse.mybir as mybir
from concourse.bass import ts
from concourse.bass_utils import run_bass_kernel_spmd
from concourse.tile import TileContext
from concourse.vector_clock import ScopedClock

N_CORES = 8
B, S, DIN, DOUT = 4, 4096, 1024, 1024
POOL, R, TOPK, NUM_TASKS = 5, 8, 3, 5
T_CORE = (B * S) // N_CORES          # 2048 tokens per core
NT = T_CORE // 128                   # 16 tiles of 128 tokens
KC = DIN // 128                      # 8 contraction chunks
NQ = 2                               # load halves (1024 tokens each, 2KB lines)
QT = T_CORE // NQ
R5 = POOL * R                        # 40 concatenated lora rows
BF16 = ml_dtypes.bfloat16
AF = mybir.ActivationFunctionType
ALU = mybir.AluOpType
BIG = 245760.0                       # 30 * 8192: exp((om - BIG)/8192) ~ 0

# ---------------------------------------------------------------------------
# Workarounds for this walrus build: at most ONE sync wait per instruction
# (zero on DmaTransposeAnt).  Excess waits are hoisted onto standalone
# EventSemaphore instructions; the Tile exit drain gets its waits emitted as
# separate wait_ge ops.
# ---------------------------------------------------------------------------

_ZERO_WAIT_OPS = {"DmaTransposeAnt"}


def _fixup_bir(bir_bytes):
    bir = _json.loads(bir_bytes)
    n = 0
    for f in bir["functions"]:
        for blk in f["blocks"]:
            out = []
            for inst in blk["instructions"]:
                si = inst.get("sync_info")
                waits = (si or {}).get("on_wait") or []
                cap = 0 if inst.get("opcode") in _ZERO_WAIT_OPS else 1
                if len(waits) > cap:
                    for w in waits[cap:]:
                        n += 1
                        out.append({
                            "debug": inst.get("debug", 0),
                            "engine": inst["engine"],
                            "ins": [], "outs": [],
                            "name": f"{inst['name']}-xw{n}",
                            "opcode": "EventSemaphore",
                            "sync_info": {"on_update": [], "on_wait": [w]},
                        })
                    si["on_wait"] = waits[:cap]
                out.append(inst)
            blk["instructions"] = out
    return _json.dumps(bir).encode()


def _install_fixup(nc):
    orig = nc.to_json_bytes
    nc.to_json_bytes = lambda: _fixup_bir(orig())
    return nc


class _TC(TileContext):
    def _drain_and_barrier(self, tick_clock, wait_clock):
        probe = self.nc.sync.drain()
        wait_clock.add_sem_waits(probe.ins, ScopedClock({None: tick_clock.global_clock}))
        waits = [(w.ant_name, w.wait_value) for w in probe.ins.sync_info.on_wait]
        probe.ins.sync_info.on_wait = []
        name2sem = {v.name: v for v in self.sems.allocated().values()}
        for nm, val in waits:
            self.nc.sync.wait_ge(name2sem[nm], val)
        self.nc.sync.drain()
        self.nc.all_engine_barrier()
        popped = self.nc._tile_sem_poison_stack.pop()
        assert popped is self._sem_poison
        self.nc.clear_and_free_semaphores(list(self.sems.allocated().values()))
        self.nc.all_engine_barrier()


# ---------------------------------------------------------------------------
# Fused kernel: y = x @ W.T (+ gated low-rank delta on the second-half tiles)
# ---------------------------------------------------------------------------

def _build_kernel(n_sliced, k):
    """n_sliced = len(omega[1:tid+1]) candidates, k = experts kept."""
    rounds = n_sliced - k                 # how many smallest candidates to drop
    nc = bass.Bass(num_devices=N_CORES)
    x_d = nc.dram_tensor("x", [DIN, T_CORE], mybir.dt.bfloat16, kind="ExternalInput")
    wt_d = nc.dram_tensor("wt", [128, KC, DOUT], mybir.dt.bfloat16, kind="ExternalInput")
    dn_d = nc.dram_tensor("dn", [128, KC, R5], mybir.dt.bfloat16, kind="ExternalInput")
    up_d = nc.dram_tensor("up", [R5, DOUT], mybir.dt.bfloat16, kind="ExternalInput")
    rt_d = nc.dram_tensor("rt", [128, KC, POOL], mybir.dt.float32, kind="ExternalInput")
    km_d = nc.dram_tensor("km", [n_sliced, R5], mybir.dt.bfloat16, kind="ExternalInput")
    y_d = nc.dram_tensor("y", [T_CORE, DOUT], mybir.dt.bfloat16, kind="ExternalOutput")

    with _TC(nc) as tc:
        with (tc.tile_pool(name="cst", bufs=1) as cpool,
              tc.tile_pool(name="io", bufs=2) as io,
              tc.tile_pool(name="ys", bufs=3) as yo,
              tc.tile_pool(name="dram", bufs=1, space="DRAM") as dram,
              tc.tile_pool(name="gp", bufs=1, space="PSUM") as gps,
              tc.tile_pool(name="pp", bufs=1, space="PSUM") as pp,
              tc.tile_pool(name="ps", bufs=2, space="PSUM") as ps):
            # ---- x first half + W interleaved on sync (earliest queue) so
            #      the PE can start tile 0 as the pieces stream in ----
            wt = cpool.tile([128, KC, DOUT], mybir.dt.bfloat16)
            xts = cpool.tile([128, KC, T_CORE], mybir.dt.bfloat16)
            for j in range(KC):
                nc.sync.dma_start(out=xts[:, j, ts(0, QT)],
                                  in_=x_d[ts(j, 128), ts(0, QT)])
                nc.sync.dma_start(out=wt[:, j, :], in_=wt_d[:, j, :])
            # ---- second half + small constants on gpsimd ----
            for j in range(KC):
                nc.gpsimd.dma_start(out=xts[:, j, ts(1, QT)],
                                    in_=x_d[ts(j, 128), ts(1, QT)])
            dn = cpool.tile([128, KC, R5], mybir.dt.bfloat16)
            nc.gpsimd.dma_start(out=dn[:], in_=dn_d[:])
            up = cpool.tile([R5, DOUT], mybir.dt.bfloat16)
            nc.gpsimd.dma_start(out=up[:], in_=up_d[:])
            rt = cpool.tile([128, KC, POOL], mybir.dt.float32)
            nc.gpsimd.dma_start(out=rt[:], in_=rt_d[:])
            km = cpool.tile([n_sliced, R5], mybir.dt.bfloat16)
            nc.gpsimd.dma_start(out=km[:], in_=km_d[:])
            one1 = cpool.tile([1, 1], mybir.dt.bfloat16)
            nc.vector.memset(one1[:], 1.0)

            # ---- partial token sums on the (otherwise idle) scalar engine;
            #      DVE stays free for the PSUM->SBUF output copies ----
            sq = cpool.tile([128, KC, NQ], mybir.dt.float32)
            scr = cpool.tile([128, QT], mybir.dt.bfloat16)
            for q in range(NQ):
                for j in range(KC):
                    nc.scalar.activation(scr[:], xts[:, j, ts(q, QT)], AF.Copy,
                                         accum_out=sq[:, j, q:q + 1])

            # ---- 8 KiB AllReduce of the piece sums (gpsimd is idle by now;
            #      halves are summed implicitly by the omega matmuls) ----
            cc_in = dram.tile([128, KC, NQ], mybir.dt.float32)
            cc_out = dram.tile([128, KC, NQ], mybir.dt.float32,
                               addr_space="Shared")
            nc.gpsimd.dma_start(out=cc_in[:], in_=sq[:])
            nc.gpsimd.collective_compute(
                "AllReduce", ALU.add,
                replica_groups=[list(range(N_CORES))],
                ins=[cc_in.opt()], outs=[cc_out.opt()],
            )
            s_tot = cpool.tile([128, KC, NQ], mybir.dt.float32)
            nc.gpsimd.dma_start(out=s_tot[:], in_=cc_out[:])

            # ---- main loop: every tile stops + copies immediately; the gated
            #      delta is applied in a short tail so PSUM never waits on the
            #      collective ----
            g = gps.tile([R5, 16], mybir.dt.float32)     # om | wT | wrep slices
            up_s = cpool.tile([R5, DOUT], mybir.dt.bfloat16)
            all_pts = []
            held_ysb = []

            for i in range(NT):
                held = i >= NT // 2  # tokens from batches {2,3}: apply delta
                y = ps.tile([128, DOUT], mybir.dt.float32, tag="y")
                if held and i % 2 == 0:
                    # down-projection for the tile PAIR (gating-independent)
                    pt = pp.tile([R5, 256], mybir.dt.float32, tag="pt")
                    for j in range(KC):
                        nc.tensor.matmul(pt[:], dn[:, j, :], xts[:, j, ts(i // 2, 256)],
                                         start=(j == 0), stop=(j == KC - 1))
                    pts = io.tile([R5, 256], mybir.dt.bfloat16, tag=f"pts{i // 2}",
                                  name=f"pts{i // 2}", bufs=1)
                    nc.vector.tensor_copy(pts[:], pt[:])
                    all_pts.append(pts)
                for j in range(KC):
                    nc.tensor.matmul(y[:, 0:512], xts[:, j, ts(i, 128)],
                                     wt[:, j, 0:512], start=(j == 0), stop=(j == KC - 1))
                    nc.tensor.matmul(y[:, 512:1024], xts[:, j, ts(i, 128)],
                                     wt[:, j, 512:1024], start=(j == 0), stop=(j == KC - 1))
                if held:
                    ysb = yo.tile([128, DOUT], mybir.dt.bfloat16, tag=f"ysbh{i}",
                                  name=f"ysbh{i}", bufs=1)
                    held_ysb.append(ysb)
                    nc.vector.tensor_copy(ysb[:], y[:])
                else:
                    ysb = yo.tile([128, DOUT], mybir.dt.bfloat16, tag="ysb")
                    nc.vector.tensor_copy(ysb[:], y[:])
                    nc.sync.dma_start(out=y_d[ts(i, 128), :], in_=ysb[:])

            # ---- gating: omega = (s_tot/BS) @ route -> top-k -> up scaling ----
            for j in range(KC):
                for q in range(NQ):
                    nc.tensor.matmul(g[0:1, 0:POOL], s_tot[:, j, q:q + 1],
                                     rt[:, j, :], start=(j == 0 and q == 0),
                                     stop=(j == KC - 1 and q == NQ - 1))
            # replicate top-k-on-sliced gating: drop `rounds` smallest
            oms = g[0:1, 1:1 + n_sliced]
            act = cpool.tile([1, n_sliced], mybir.dt.float32)
            if rounds:
                excl = cpool.tile([1, n_sliced], mybir.dt.float32)
                nc.vector.memset(excl[:], 0.0)
                v = cpool.tile([1, n_sliced], mybir.dt.float32)
                nc.vector.tensor_copy(v[:], oms)
                for _ in range(rounds):
                    mn = cpool.tile([1, 1], mybir.dt.float32, tag="mn")
                    nc.vector.tensor_reduce(out=mn[:], in_=v[:],
                                            axis=mybir.AxisListType.X, op=ALU.min)
                    mk = cpool.tile([1, n_sliced], mybir.dt.float32, tag="mk")
                    nc.vector.tensor_scalar(out=mk[:], in0=v[:], scalar1=mn[:],
                                            scalar2=BIG, op0=ALU.is_equal,
                                            op1=ALU.mult)     # (v==mn)*BIG
                    nc.vector.tensor_tensor(out=excl[:], in0=excl[:], in1=mk[:],
                                            op=ALU.add)
                    nc.vector.tensor_tensor(out=v[:], in0=v[:], in1=mk[:],
                                            op=ALU.add)
                nc.vector.tensor_tensor(out=act[:], in0=oms, in1=excl[:],
                                        op=ALU.subtract)
            else:
                nc.vector.tensor_copy(act[:], oms)
            # softmax over the survivors (dropped ones -> exp(-30) ~ 0);
            # Exp+row-sum fused via accum_out
            e = cpool.tile([1, n_sliced], mybir.dt.float32)
            ssum = cpool.tile([1, 1], mybir.dt.float32)
            nc.scalar.activation(e[:], act[:], AF.Exp, scale=1.0 / float(B * S),
                                 accum_out=ssum[:])
            rs = cpool.tile([1, 1], mybir.dt.float32)
            nc.vector.reciprocal(rs[:], ssum[:])
            w = cpool.tile([1, n_sliced], mybir.dt.bfloat16)
            nc.vector.tensor_scalar(out=w[:], in0=e[:], scalar1=rs[:],
                                    scalar2=None, op0=ALU.mult)
            # wrep[40,1] = Kmat.T @ w.T ; fold into lora_up rows
            nc.tensor.matmul(g[0:n_sliced, 8:9], w[:], one1[:],
                             start=True, stop=True)
            wt4 = cpool.tile([n_sliced, 1], mybir.dt.bfloat16)
            nc.vector.tensor_copy(wt4[:], g[0:n_sliced, 8:9])
            nc.tensor.matmul(g[0:R5, 9:10], km[:], wt4[:], start=True, stop=True)
            wrep = cpool.tile([R5, 1], mybir.dt.float32)
            nc.vector.tensor_copy(wrep[:], g[0:R5, 9:10])
            nc.scalar.activation(up_s[:], up[:], AF.Copy, scale=wrep[:])

            # ---- delta tail: rank-40 update added into the staged outputs ----
            for i in range(NT // 2, NT):
                pts = all_pts[(i - NT // 2) // 2]
                ph = pts[:, ts(i % 2, 128)]
                ysb = held_ysb[i - NT // 2]
                for h in range(2):
                    pd = pp.tile([128, 512], mybir.dt.float32, tag="pd", bufs=2)
                    nc.tensor.matmul(pd[:], ph, up_s[:, ts(h, 512)],
                                     start=True, stop=True)
                    nc.vector.tensor_tensor(out=ysb[:, ts(h, 512)], in0=pd[:],
                                            in1=ysb[:, ts(h, 512)], op=ALU.add)
                nc.sync.dma_start(out=y_d[ts(i, 128), :], in_=ysb[:])
    return _install_fixup(nc)


_NC_CACHE = {}


def _get_nc(n_sliced, k):
    key = (n_sliced, k)
    if key not in _NC_CACHE:
        _NC_CACHE[key] = _build_kernel(n_sliced, k)
    return _NC_CACHE[key]


LAST_RESULTS = {}  # test-harness hook: BassKernelResults of the last call


def kernel(input, W, lora_down, lora_up, lora_route, task_id):
    x = np.ascontiguousarray(np.asarray(input, dtype=np.float32)).reshape(B * S, DIN)
    W = np.asarray(W, dtype=np.float32)
    lora_down = np.asarray(lora_down, dtype=np.float32)
    lora_up = np.asarray(lora_up, dtype=np.float32)
    lora_route = np.asarray(lora_route, dtype=np.float32)
    tid = min(int(task_id), NUM_TASKS)
    k = min(tid, TOPK)

    half = (B * S) // 2
    per = half // N_CORES  # 1024 tokens from each half per core
    xT = np.ascontiguousarray(x.T).astype(BF16)  # [DIN, B*S]; the bf16 cast
    # replaces the cast-DMA the device would otherwise do on the same values
    shards = [np.concatenate([xT[:, c * per:(c + 1) * per],
                              xT[:, half + c * per:half + (c + 1) * per]], axis=1)
              for c in range(N_CORES)]

    down_cat = lora_down.transpose(1, 0, 2).reshape(DIN, R5)
    wt_h = np.ascontiguousarray(W.T.reshape(KC, 128, DOUT).transpose(1, 0, 2)).astype(BF16)
    dn_h = np.ascontiguousarray(down_cat.reshape(KC, 128, R5).transpose(1, 0, 2)).astype(BF16)
    up_h = lora_up.reshape(R5, DOUT).astype(BF16)
    rt_h = np.ascontiguousarray(
        lora_route[1].reshape(KC, 128, POOL).transpose(1, 0, 2)).astype(np.float32)
    km_h = np.zeros((tid, R5), np.float32)
    for p in range(min(tid, POOL)):
        km_h[p, p * R:(p + 1) * R] = 1.0  # sliced position p -> expert p

    in_maps = [{"x": s, "wt": wt_h, "dn": dn_h, "up": up_h,
                "rt": rt_h, "km": km_h.astype(BF16)} for s in shards]
    res = run_bass_kernel_spmd(_get_nc(tid, k), in_maps, list(range(N_CORES)))
    LAST_RESULTS["b"] = res

    y = np.empty((B * S, DOUT), np.float32)
    for c in range(N_CORES):
        yc = res.results[c]["y"].astype(np.float32)
        y[c * per:(c + 1) * per] = yc[:per]
        y[half + c * per:half + (c + 1) * per] = yc[per:]
    return y.reshape(B, S, DOUT)
